# Initial kernel scaffold
#
"""DGCNN hypergraph kernel for Trainium2 (Bass/Tile), 8-core SPMD.

Strategy (per the data-parallel sharding hint): 128 disjoint hypergraphs are
sharded 16-per-core across 8 NeuronCores. All message passing is graph-local.

Per-core pipeline (16 graphs, processed as 4 groups of 4):
  - A-build: per-graph 512x512 incidence-count matrix A (fp16, counts exact)
    via multiplicity computation (eq-matrix + tree reduce) and gpsimd
    local_scatter (duplicate slots write identical values).
  - At = A^T via TensorE block transposes; per-edge sizes via ones-matmul
    (replicated across partitions), then +1 and reciprocal.
  - 4 conv layers x 2 directions: linear (fp32 matmul, block-diag weights for
    4-graph batching), PE transpose to node-major, fp16 hi/lo pair split, and
    aggregation as col-tiled fp16 matmuls against A / At accumulated in PSUM
    (2-pass hi/lo gives ~fp32 accuracy), then bias/degree-scale + tanh.
  - Sort-pooling: top-30 per graph via max8/max_index/match_replace rounds
    (tie behavior matches jax stable top_k), gather pooled rows via ap_gather.
  - Conv tower + dense layer via small fp32 matmuls, relu, output assembly.
"""

import numpy as np
from contextlib import ExitStack

import concourse.bass as bass
import concourse.tile as tile
from concourse import bacc, mybir
from concourse.bass_utils import run_bass_kernel_spmd

dt = mybir.dt
ALU = mybir.AluOpType
AF = mybir.ActivationFunctionType
AX = mybir.AxisListType

B = 128          # graphs
NPER = 512       # nodes per graph
EPER = 512       # hyperedges per graph
DEG = 32         # memberships per node
F = 128          # input feature dim
K = 30           # sortpool k
NCORES = 8
GPC = B // NCORES          # 16 graphs per core
NGROUP = GPC // 4          # 4 groups of 4 graphs
C1, C2, KW2 = 16, 32, 5
HDEG = float(DEG + 1)      # node hyperdegree + 1 (structural: 33)

_CACHE = {}


def _pad32(w):
    """Zero-pad a [din, dout] weight to [32, 32]."""
    out = np.zeros((32, 32), np.float32)
    out[: w.shape[0], : w.shape[1]] = w
    return out


def _blockdiag4(w):
    """[32, 32] -> [128, 128] block-diagonal (4 copies)."""
    out = np.zeros((128, 128), np.float32)
    for g in range(4):
        out[32 * g : 32 * g + 32, 32 * g : 32 * g + 32] = w
    return out


def _build_program():
    nc = bacc.Bacc("TRN2", target_bir_lowering=False, debug=False,
                   num_devices=NCORES)

    # ---- DRAM I/O ----
    NF = nc.dram_tensor("nf", [GPC, NPER, F], dt.float32, kind="ExternalInput").ap()
    EINC = nc.dram_tensor("einc", [GPC, NPER, DEG], dt.int16, kind="ExternalInput").ap()
    # fp32 constants
    W0 = nc.dram_tensor("w0c", [128, 32], dt.float32, kind="ExternalInput").ap()
    BDE = nc.dram_tensor("bde", [3, 128, 128], dt.float32, kind="ExternalInput").ap()   # E-side blockdiag l=1..3
    BDN = nc.dram_tensor("bdn", [4, 128, 128], dt.float32, kind="ExternalInput").ap()   # N-side blockdiag l=0..3
    BEPP = nc.dram_tensor("bepp", [4, 128, 1], dt.float32, kind="ExternalInput").ap()   # edge-side bias per (g,f)
    BNPP = nc.dram_tensor("bnpp", [4, 128, 1], dt.float32, kind="ExternalInput").ap()   # node-side bias/33 per (g,f)
    IDENT = nc.dram_tensor("ident", [128, 128], dt.float32, kind="ExternalInput").ap()
    IDENT16 = nc.dram_tensor("ident16", [128, 128], dt.float16, kind="ExternalInput").ap()
    ONES16 = nc.dram_tensor("ones16", [128, 32], dt.float16, kind="ExternalInput").ap()
    CW1 = nc.dram_tensor("cw1", [4, 128, 16], dt.float32, kind="ExternalInput").ap()
    CB1 = nc.dram_tensor("cb1", [128, 1], dt.float32, kind="ExternalInput").ap()
    CW2 = nc.dram_tensor("cw2", [5, 128, 32], dt.float32, kind="ExternalInput").ap()
    CB2 = nc.dram_tensor("cb2", [128, 1], dt.float32, kind="ExternalInput").ap()
    OW = nc.dram_tensor("ow", [2, 128, 11], dt.float32, kind="ExternalInput").ap()
    OUTB = nc.dram_tensor("outb", [4, 8], dt.float32, kind="ExternalInput").ap()
    SSUM = nc.dram_tensor("ssum", [128, 4], dt.float32, kind="ExternalInput").ap()
    OUT = nc.dram_tensor("out", [GPC, 2], dt.float32, kind="ExternalOutput").ap()
    IDXS = nc.dram_tensor("idxscratch", [GPC, 32], dt.int16, kind="Internal").ap()

    with tile.TileContext(nc) as tc, ExitStack() as ctx:
        cpool = ctx.enter_context(tc.tile_pool(name="consts", bufs=1))
        gpool = ctx.enter_context(tc.tile_pool(name="graph", bufs=2))
        apool = ctx.enter_context(tc.tile_pool(name="amat", bufs=2))
        hpool = ctx.enter_context(tc.tile_pool(name="acts", bufs=2))
        hcatp = ctx.enter_context(tc.tile_pool(name="hcat", bufs=4))
        tpool = ctx.enter_context(tc.tile_pool(name="tmp", bufs=2))
        kpool = ctx.enter_context(tc.tile_pool(name="keys", bufs=1))
        ps = ctx.enter_context(tc.tile_pool(name="ps", bufs=2, space="PSUM"))
        ps2 = ctx.enter_context(tc.tile_pool(name="ps2", bufs=2, space="PSUM"))
        # bank budget (8): ps{mm1,agg,aux} x2 = 6, ps2{small} x2 = 2

        def cload(name, src, shape, dtype):
            t = cpool.tile(shape, dtype, tag=name)
            nc.sync.dma_start(t[:], src)
            return t

        w0 = cload("w0", W0, [128, 32], dt.float32)
        bde = [cload(f"bde{l}", BDE[l], [128, 128], dt.float32) for l in range(3)]
        bdn = [cload(f"bdn{l}", BDN[l], [128, 128], dt.float32) for l in range(4)]
        bepp = [cload(f"bepp{l}", BEPP[l], [128, 1], dt.float32) for l in range(4)]
        bnpp = [cload(f"bnpp{l}", BNPP[l], [128, 1], dt.float32) for l in range(4)]
        ident = cload("ident", IDENT, [128, 128], dt.float32)
        ident16 = cload("ident16", IDENT16, [128, 128], dt.float16)
        ones16 = cload("ones16", ONES16, [128, 32], dt.float16)
        cw1 = [cload(f"cw1{l}", CW1[l], [128, 16], dt.float32) for l in range(4)]
        cb1 = cload("cb1", CB1, [128, 1], dt.float32)
        cw2 = [cload(f"cw2{d}", CW2[d], [128, 32], dt.float32) for d in range(5)]
        cb2 = cload("cb2", CB2, [128, 1], dt.float32)
        ow = [cload(f"ow{o}", OW[o], [128, 11], dt.float32) for o in range(2)]
        outb = cload("outb", OUTB, [4, 8], dt.float32)
        ssum = cload("ssum", SSUM, [128, 4], dt.float32)

        keys16 = kpool.tile([16, 512], dt.float32, tag="keys16")
        Yout = kpool.tile([128, 8], dt.float32, tag="yout")
        idx16 = kpool.tile([16, 32], dt.int16, tag="idx16")

        h4T_all = []   # per-group layer-3 output tiles (for keys)

        def build_A(g_local, g, eng):
            """Build A [128,(4c),512] fp16 + At + recip for one graph.
            eng: 'v' (DVE) or 'g' (gpsimd) for the eq/tree work."""
            ve = nc.vector if eng == "v" else nc.gpsimd
            einc = gpool.tile([128, 4, 32], dt.int16, tag="einc")
            src = EINC[g_local].rearrange("(c p) k -> p c k", p=128)
            nc.sync.dma_start(einc[:], src)

            einf = gpool.tile([128, 4, 32], dt.float16, tag="einf")
            nc.vector.tensor_copy(einf[:], einc[:])
            eqm = tpool.tile([128, 4, 32, 32], dt.float16, tag="eqm")
            e_rep = einf[:].unsqueeze(3).broadcast_to([128, 4, 32, 32])
            e_til = einf[:].unsqueeze(2).broadcast_to([128, 4, 32, 32])
            ve.tensor_tensor(eqm[:], e_rep, e_til, ALU.is_equal)
            # tree reduce over l (last dim 32 -> 1)
            t16 = tpool.tile([128, 4, 32, 16], dt.float16, tag="t16")
            ve.tensor_tensor(t16[:], eqm[:, :, :, 0:16], eqm[:, :, :, 16:32], ALU.add)
            t8 = tpool.tile([128, 4, 32, 8], dt.float16, tag="t8")
            ve.tensor_tensor(t8[:], t16[:, :, :, 0:8], t16[:, :, :, 8:16], ALU.add)
            t4 = tpool.tile([128, 4, 32, 4], dt.float16, tag="t4")
            ve.tensor_tensor(t4[:], t8[:, :, :, 0:4], t8[:, :, :, 4:8], ALU.add)
            t2 = tpool.tile([128, 4, 32, 2], dt.float16, tag="t2")
            ve.tensor_tensor(t2[:], t4[:, :, :, 0:2], t4[:, :, :, 2:4], ALU.add)
            mf = tpool.tile([128, 4, 32], dt.float16, tag="mf")
            ve.tensor_tensor(mf[:], t2[:, :, :, 0], t2[:, :, :, 1], ALU.add)

            A = apool.tile([128, 4, 512], dt.float16, tag=f"A{g}")
            for c in range(4):
                nc.gpsimd.local_scatter(A[:, c, :], mf[:, c, :], einc[:, c, :],
                                        channels=128, num_elems=512, num_idxs=32)
            # At via PE block transposes (fp16)
            At = apool.tile([128, 4, 512], dt.float16, tag=f"At{g}")
            for ce in range(4):
                ptr = ps2.tile([128, 512], dt.float16, tag="small")
                for cn in range(4):
                    nc.tensor.transpose(ptr[:, 128 * cn : 128 * cn + 128],
                                        A[:, cn, 128 * ce : 128 * ce + 128],
                                        ident16[:])
                eng2 = nc.scalar if ce % 2 == 0 else nc.vector
                if ce % 2 == 0:
                    nc.scalar.copy(At[:, ce, :], ptr[:])
                else:
                    nc.vector.tensor_copy(At[:, ce, :], ptr[:])
            return A, At

        for G in range(NGROUP):
            As, Ats = [], []
            for g in range(4):
                gl = 4 * G + g
                A, At = build_A(gl, g, "v")
                As.append(A)
                Ats.append(At)

            # hsize (replicated over each 32-partition block) + reciprocal
            hsz = ps.tile([128, 512], dt.float32, tag="aux")
            for g in range(4):
                for c in range(4):
                    nc.tensor.matmul(hsz[32 * g : 32 * g + 32, :], ones16[:],
                                     As[g][:, c, :], start=(c == 0), stop=(c == 3),
                                     tile_position=(0, 32 * g))
            hsp = tpool.tile([128, 512], dt.float32, tag="hsp")
            nc.vector.tensor_scalar_add(hsp[:], hsz[:], 1.0)
            recip = hpool.tile([128, 512], dt.float32, tag="recip")
            nc.vector.reciprocal(recip[:], hsp[:])

            # layer-0 input: node features transposed per graph
            h0Ts = []
            for g in range(4):
                gl = 4 * G + g
                nf_t = gpool.tile([128, 4, 128], dt.float32, tag="nf")
                nc.sync.dma_start(nf_t[:], NF[gl].rearrange("(c p) f -> p c f", p=128))
                pnf = ps.tile([128, 512], dt.float32, tag="aux")
                for c in range(4):
                    nc.tensor.transpose(pnf[:, 128 * c : 128 * c + 128],
                                        nf_t[:, c, :], ident[:])
                h0T = hpool.tile([128, 512], dt.float32, tag=f"h0T{g}")
                if g % 2 == 0:
                    nc.scalar.copy(h0T[:], pnf[:])
                else:
                    nc.vector.tensor_copy(h0T[:], pnf[:])
                h0Ts.append(h0T)

            hT = None   # group activation tile [128(4g x 32f), 512]
            for l in range(4):
                # ---------- direction E: node -> hyperedge ----------
                zT = ps.tile([128, 512], dt.float32, tag="mm1")
                if l == 0:
                    for g in range(4):
                        nc.tensor.matmul(zT[32 * g : 32 * g + 32, :], w0[:],
                                         h0Ts[g][:], start=True, stop=True,
                                         tile_position=(0, 32 * g))
                else:
                    nc.tensor.matmul(zT[:], bde[l - 1][:], hT[:], start=True, stop=True)
                zTs = tpool.tile([128, 512], dt.float32, tag="zTs")
                nc.scalar.copy(zTs[:], zT[:])
                zN = ps.tile([128, 512], dt.float32, tag="mm1")
                for c in range(4):
                    nc.tensor.transpose(zN[:, 128 * c : 128 * c + 128],
                                        zTs[:, 128 * c : 128 * c + 128], ident[:])
                zhi = tpool.tile([128, 512], dt.float16, tag="zhi")
                nc.scalar.copy(zhi[:], zN[:])
                zlo = tpool.tile([128, 512], dt.float16, tag="zlo")
                nc.vector.tensor_tensor(zlo[:], zN[:], zhi[:], ALU.subtract)

                agg = ps.tile([128, 512], dt.float32, tag="agg")
                for g in range(4):
                    n = 0
                    for zp in (zhi, zlo):
                        for c in range(4):
                            nc.tensor.matmul(
                                agg[32 * g : 32 * g + 32, :],
                                zp[:, 128 * c + 32 * g : 128 * c + 32 * g + 32],
                                As[g][:, c, :], start=(n == 0), stop=(n == 7),
                                tile_position=(0, 32 * g))
                            n += 1
                ue = tpool.tile([128, 512], dt.float32, tag="ue")
                nc.vector.scalar_tensor_tensor(ue[:], agg[:], bepp[l][:], recip[:],
                                               ALU.add, ALU.mult)
                heT = hpool.tile([128, 512], dt.float32, tag="heT")
                nc.scalar.activation(heT[:], ue[:], AF.Tanh)

                # ---------- direction N: hyperedge -> node ----------
                vT = ps.tile([128, 512], dt.float32, tag="mm1")
                nc.tensor.matmul(vT[:], bdn[l][:], heT[:], start=True, stop=True)
                vTs = tpool.tile([128, 512], dt.float32, tag="vTs")
                nc.vector.tensor_copy(vTs[:], vT[:])
                vN = ps.tile([128, 512], dt.float32, tag="mm1")
                for c in range(4):
                    nc.tensor.transpose(vN[:, 128 * c : 128 * c + 128],
                                        vTs[:, 128 * c : 128 * c + 128], ident[:])
                vhi = tpool.tile([128, 512], dt.float16, tag="zhi")
                nc.scalar.copy(vhi[:], vN[:])
                vlo = tpool.tile([128, 512], dt.float16, tag="zlo")
                nc.vector.tensor_tensor(vlo[:], vN[:], vhi[:], ALU.subtract)

                aggn = ps.tile([128, 512], dt.float32, tag="agg")
                for g in range(4):
                    n = 0
                    for vp in (vhi, vlo):
                        for c in range(4):
                            nc.tensor.matmul(
                                aggn[32 * g : 32 * g + 32, :],
                                vp[:, 128 * c + 32 * g : 128 * c + 32 * g + 32],
                                Ats[g][:, c, :], start=(n == 0), stop=(n == 7),
                                tile_position=(0, 32 * g))
                            n += 1
                hT = hcatp.tile([128, 512], dt.float32, tag=f"hT{l}")
                nc.scalar.activation(hT[:], aggn[:], AF.Tanh, bias=bnpp[l][:],
                                     scale=1.0 / HDEG)
                if l == 3:
                    h4T_all.append(hT)
                    # keys: feature-0 row of each graph block
                    krows = hT[:].rearrange("(a b) f -> a b f", b=32)[:, 0, :]
                    nc.sync.dma_start(keys16[4 * G : 4 * G + 4, :], krows)
                if l == 0:
                    hcat0 = hT
                elif l == 1:
                    hcat1 = hT
                elif l == 2:
                    hcat2 = hT

            # stash for pooled gather
            if G == 0:
                hcats_all = []
            hcats_all.append((hcat0, hcat1, hcat2, h4T_all[G]))

        # ---------- top-k (all 16 graphs at once) ----------
        kw = kpool.tile([16, 512], dt.float32, tag="kw")
        nc.vector.tensor_copy(kw[:], keys16[:])
        idxu = kpool.tile([16, 32], dt.uint32, tag="idxu")
        for r in range(4):
            m8 = kpool.tile([16, 8], dt.float32, tag="m8")
            nc.vector.max(m8[:], kw[:])
            nc.vector.max_index(idxu[:, 8 * r : 8 * r + 8], m8[:], kw[:])
            nc.vector.match_replace(kw[:], m8[:], kw[:], -1e30)
        nc.vector.tensor_copy(idx16[:], idxu[:])
        nc.sync.dma_start(IDXS, idx16[:])

        # ---------- pooled gather + conv tower per group ----------
        for G in range(NGROUP):
            tiles = hcats_all[G]
            idxw = tpool.tile([128, 2], dt.int16, tag="idxw")
            for m in range(4):
                src_m = IDXS[4 * G + m].rearrange("(t lo) -> lo t", lo=16)
                for half in range(2):
                    base = 32 * m + 16 * half
                    nc.sync.dma_start(idxw[base : base + 16, :], src_m)

            pgs = []
            for l in range(4):
                pg = tpool.tile([128, 32], dt.float32, tag=f"pg{l}")
                nc.gpsimd.ap_gather(pg[:], tiles[l][:].unsqueeze(2), idxw[:],
                                    channels=128, num_elems=512, d=1, num_idxs=32)
                pgs.append(pg)

            y1 = ps2.tile([128, 30], dt.float32, tag="small")
            for g in range(4):
                for l in range(4):
                    nc.tensor.matmul(y1[32 * g : 32 * g + 16, :],
                                     cw1[l][32 * g : 32 * g + 32, :],
                                     pgs[l][32 * g : 32 * g + 32, 0:30],
                                     start=(l == 0), stop=(l == 3),
                                     tile_position=(32 * g, 32 * g))
            y1r = tpool.tile([128, 30], dt.float32, tag="y1r")
            nc.scalar.activation(y1r[:], y1[:], AF.Relu, bias=cb1[:])
            y1p = tpool.tile([128, 15], dt.float32, tag="y1p")
            nc.vector.tensor_tensor(
                y1p[:], y1r[:].rearrange("p (t two) -> p t two", two=2)[:, :, 0],
                y1r[:].rearrange("p (t two) -> p t two", two=2)[:, :, 1], ALU.max)

            y2 = ps2.tile([128, 11], dt.float32, tag="small")
            for g in range(4):
                for d in range(5):
                    nc.tensor.matmul(y2[32 * g : 32 * g + 32, :],
                                     cw2[d][32 * g : 32 * g + 32, :],
                                     y1p[32 * g : 32 * g + 32, d : d + 11],
                                     start=(d == 0), stop=(d == 4),
                                     tile_position=(32 * g, 32 * g))
            y2r = tpool.tile([128, 11], dt.float32, tag="y2r")
            nc.scalar.activation(y2r[:], y2[:], AF.Relu, bias=cb2[:])
            for o in range(2):
                t_o = tpool.tile([128, 11], dt.float32, tag="t_o")
                nc.vector.tensor_tensor(t_o[:], y2r[:], ow[o][:], ALU.mult)
                nc.vector.tensor_reduce(Yout[:, 2 * G + o : 2 * G + o + 1], t_o[:],
                                        AX.X, ALU.add)

        # ---------- final dense + relu + output ----------
        pout = ps2.tile([4, 8], dt.float32, tag="small")
        nc.tensor.matmul(pout[:], ssum[:], Yout[:], start=True, stop=True)
        ob = kpool.tile([4, 8], dt.float32, tag="ob")
        nc.vector.tensor_tensor(ob[:], pout[:], outb[:], ALU.add)
        orl = kpool.tile([4, 8], dt.float32, tag="orl")
        nc.scalar.activation(orl[:], ob[:], AF.Relu)
        nc.sync.dma_start(OUT.rearrange("(G g) o -> g G o", g=4), orl[:])

    nc.compile()
    return nc


def _make_consts(inputs):
    ws = [inputs[f"w{i}"].astype(np.float32) for i in range(8)]
    bs = [inputs[f"b{i}"].astype(np.float32) for i in range(8)]
    wE = [ws[0], _pad32(ws[2]), _pad32(ws[4]), _pad32(ws[6])]
    wN = [_pad32(ws[1]), _pad32(ws[3]), _pad32(ws[5]), _pad32(ws[7])]
    bE = [np.pad(bs[0], (0, 0)), np.pad(bs[2], (0, 0)), np.pad(bs[4], (0, 0)),
          np.pad(bs[6], (0, 31))]
    bN = [bs[1], bs[3], bs[5], np.pad(bs[7], (0, 31))]

    bde = np.stack([_blockdiag4(wE[l]) for l in range(1, 4)])
    bdn = np.stack([_blockdiag4(wN[l]) for l in range(4)])
    bepp = np.stack([np.tile(bE[l], 4)[:, None] for l in range(4)]).astype(np.float32)
    bnpp = np.stack([(np.tile(bN[l], 4) / HDEG)[:, None] for l in range(4)]).astype(np.float32)

    c1w = inputs["conv1_w"].astype(np.float32).reshape(C1, 97)    # [16, 97]
    cw1 = np.zeros((4, 128, 16), np.float32)
    for l in range(4):
        blk = np.zeros((32, 16), np.float32)
        if l < 3:
            blk = c1w[:, 32 * l : 32 * l + 32].T
        else:
            blk[0, :] = c1w[:, 96]
        for g in range(4):
            cw1[l, 32 * g : 32 * g + 32, :] = blk
    cb1 = np.zeros((128,), np.float32)
    for g in range(4):
        cb1[32 * g : 32 * g + 16] = inputs["conv1_b"]
    c2w = inputs["conv2_w"].astype(np.float32)                    # [32, 16, 5]
    cw2 = np.zeros((5, 128, 32), np.float32)
    for d in range(5):
        for g in range(4):
            cw2[d, 32 * g : 32 * g + 16, :] = c2w[:, :, d].T      # upper 16 of block zero
    cb2 = np.zeros((128,), np.float32)
    for g in range(4):
        cb2[32 * g : 32 * g + 32] = inputs["conv2_b"]
    oww = inputs["out_w"].astype(np.float32)                      # [352, 2]
    ow = np.zeros((2, 128, 11), np.float32)
    for o in range(2):
        for g in range(4):
            ow[o, 32 * g : 32 * g + 32, :] = oww[:, o].reshape(C2, 11)
    outb = np.tile(inputs["out_b"].astype(np.float32), (4, 4))    # [4, 8]
    ssum = np.zeros((128, 4), np.float32)
    for j in range(4):
        ssum[32 * j : 32 * j + 32, j] = 1.0

    return {
        "w0c": ws[0],
        "bde": bde, "bdn": bdn, "bepp": bepp, "bnpp": bnpp,
        "ident": np.eye(128, dtype=np.float32),
        "ident16": np.eye(128, dtype=np.float16),
        "ones16": np.ones((128, 32), np.float16),
        "cw1": cw1, "cb1": cb1[:, None], "cw2": cw2, "cb2": cb2[:, None],
        "ow": ow, "outb": outb, "ssum": ssum,
    }


def get_program():
    if "nc" not in _CACHE:
        _CACHE["nc"] = _build_program()
    return _CACHE["nc"]


def make_in_maps(inputs):
    consts = _make_consts(inputs)
    nf = np.ascontiguousarray(inputs["node_feat"].astype(np.float32).reshape(B, NPER, F))
    einc_g = np.asarray(inputs["inc_edge"]).reshape(B, NPER, DEG)
    base = (np.arange(B, dtype=np.int64) * EPER)[:, None, None]
    einc_l = (einc_g - base).astype(np.int16)
    in_maps = []
    for c in range(NCORES):
        m = dict(consts)
        m["nf"] = np.ascontiguousarray(nf[c * GPC : (c + 1) * GPC])
        m["einc"] = np.ascontiguousarray(einc_l[c * GPC : (c + 1) * GPC])
        in_maps.append(m)
    return in_maps


def kernel(**inputs):
    nc = get_program()
    in_maps = make_in_maps(inputs)
    res = run_bass_kernel_spmd(nc, in_maps, core_ids=list(range(NCORES)))
    out = np.concatenate([res.results[c]["out"] for c in range(NCORES)], axis=0)
    return out.astype(np.float32)



# revision 25
# speedup vs baseline: 5074.1320x; 5074.1320x over previous
"""DGCNN hypergraph kernel for Trainium2 (Bass/Tile), 8-core SPMD.

128 disjoint hypergraphs sharded 16-per-core across 8 NeuronCores,
processed as 4 groups of 4 graphs; groups run PAIR-INTERLEAVED so the
serial per-direction chain (PE mm -> split -> PE agg -> split -> PE SEL
-> DVE/ACT) of one group overlaps the other group's chain on idle
engines, keeping the PE warm (HAM clock) and queues non-blocking.

Host precomputes: dense incidence A [128p, 4c, 512e] and its transpose
in fp8e4m3 (counts <= ~6, exact; fp8 moving x fp16 stationary measured
exact), feature-major node features fp32, per-edge 1/(hsize+1) fp32.

Per direction: linear via "W-as-moving" fp32 matmuls (stationary =
activation slices, moving = block-diag W) -> node-major z, zero PE
transposes; z split into packed 64-col (hi|lo) fp16 stationary; ONE
fp8 pass of A per graph accumulating [64, 512]; hi+lo PSUM halves
combined by a selection matmul over an exact 3-component fp16
decomposition (engines cannot cross partitions); (u+b)*recip / tanh.
Sort-pooling (top-30) + conv tower run per pair to overlap the tail.
"""

import numpy as np
from contextlib import ExitStack

import ml_dtypes
import concourse.bass as bass
import concourse.tile as tile
from concourse import bacc, mybir
from concourse.bass_utils import run_bass_kernel_spmd

dt = mybir.dt
ALU = mybir.AluOpType
AF = mybir.ActivationFunctionType
AX = mybir.AxisListType

B = 128
NPER = 512
EPER = 512
DEG = 32
F = 128
K = 30
NCORES = 8
GPC = B // NCORES          # 16 graphs per core
NGROUP = GPC // 4          # 4 groups of 4 graphs
C1, C2, KW2 = 16, 32, 5
HDEG = float(DEG + 1)

_CACHE = {}


def _pad32(w):
    out = np.zeros((32, 32), np.float32)
    out[: w.shape[0], : w.shape[1]] = w
    return out


def _blockdiag4(w):
    out = np.zeros((128, 128), np.float32)
    for g in range(4):
        out[32 * g : 32 * g + 32, 32 * g : 32 * g + 32] = w
    return out


def _build_program():
    nc = bacc.Bacc("TRN2", target_bir_lowering=False, debug=False,
                   num_devices=NCORES)

    AD = nc.dram_tensor("amat", [GPC, 128, 4, 512], dt.float8e4, kind="ExternalInput").ap()
    ATD = nc.dram_tensor("atmat", [GPC, 128, 4, 512], dt.float8e4, kind="ExternalInput").ap()
    NFT = nc.dram_tensor("nft", [GPC, 128, 512], dt.float32, kind="ExternalInput").ap()
    RECIP = nc.dram_tensor("recip", [NGROUP, 128, 512], dt.float32, kind="ExternalInput").ap()
    W0D = nc.dram_tensor("w0c", [128, 32], dt.float32, kind="ExternalInput").ap()
    BDE = nc.dram_tensor("bde", [3, 128, 128], dt.float32, kind="ExternalInput").ap()
    BDN = nc.dram_tensor("bdn", [4, 128, 128], dt.float32, kind="ExternalInput").ap()
    BEPP = nc.dram_tensor("bepp", [4, 128, 1], dt.float32, kind="ExternalInput").ap()
    BNPP = nc.dram_tensor("bnpp", [4, 128, 1], dt.float32, kind="ExternalInput").ap()
    SELD = nc.dram_tensor("sel", [128, 64], dt.float16, kind="ExternalInput").ap()
    CW1 = nc.dram_tensor("cw1", [4, 128, 16], dt.float32, kind="ExternalInput").ap()
    CB1 = nc.dram_tensor("cb1", [128, 1], dt.float32, kind="ExternalInput").ap()
    CW2 = nc.dram_tensor("cw2", [5, 128, 32], dt.float32, kind="ExternalInput").ap()
    CB2 = nc.dram_tensor("cb2", [128, 1], dt.float32, kind="ExternalInput").ap()
    OW = nc.dram_tensor("ow", [2, 128, 11], dt.float32, kind="ExternalInput").ap()
    OUTB = nc.dram_tensor("outb", [4, 8], dt.float32, kind="ExternalInput").ap()
    SSUM = nc.dram_tensor("ssum", [128, 4], dt.float32, kind="ExternalInput").ap()
    OUT = nc.dram_tensor("out", [GPC, 2], dt.float32, kind="ExternalOutput").ap()
    IDXS = nc.dram_tensor("idxscratch", [GPC, 32], dt.int16, kind="Internal").ap()

    with tile.TileContext(nc) as tc, ExitStack() as ctx:
        cpool = ctx.enter_context(tc.tile_pool(name="consts", bufs=1))
        apool = ctx.enter_context(tc.tile_pool(name="amat", bufs=2))
        gpool = ctx.enter_context(tc.tile_pool(name="graph", bufs=2))
        spool = ctx.enter_context(tc.tile_pool(name="scratch", bufs=2))
        hpool = ctx.enter_context(tc.tile_pool(name="hstash", bufs=4))
        kpool = ctx.enter_context(tc.tile_pool(name="keys", bufs=1))
        tpool = ctx.enter_context(tc.tile_pool(name="tmp", bufs=2))
        ps = ctx.enter_context(tc.tile_pool(name="ps", bufs=1, space="PSUM"))
        # 8 PSUM banks: zmm0/1, u0/1, a01_0/1, a23_0/1 (pooling reuses u*)

        def cload(name, src, shape, dtype):
            t = cpool.tile(shape, dtype, tag=name, name=name)
            nc.sync.dma_start(t[:], src)
            return t

        w0 = cload("w0", W0D, [128, 32], dt.float32)
        bde = [cload(f"bde{l}", BDE[l], [128, 128], dt.float32) for l in range(3)]
        bdn = [cload(f"bdn{l}", BDN[l], [128, 128], dt.float32) for l in range(4)]
        bepp = [cload(f"bepp{l}", BEPP[l], [128, 1], dt.float32) for l in range(4)]
        bnpp = [cload(f"bnpp{l}", BNPP[l], [128, 1], dt.float32) for l in range(4)]
        sel = cload("sel", SELD, [128, 64], dt.float16)
        cw1 = [cload(f"cw1{l}", CW1[l], [128, 16], dt.float32) for l in range(4)]
        cb1 = cload("cb1", CB1, [128, 1], dt.float32)
        cw2 = [cload(f"cw2{d}", CW2[d], [128, 32], dt.float32) for d in range(5)]
        cb2 = cload("cb2", CB2, [128, 1], dt.float32)
        ow = [cload(f"ow{o}", OW[o], [128, 11], dt.float32) for o in range(2)]
        outb = cload("outb", OUTB, [4, 8], dt.float32)
        ssum = cload("ssum", SSUM, [128, 4], dt.float32)

        keys8 = [kpool.tile([8, 512], dt.float32, tag=f"keys{P}", name=f"keys{P}")
                 for P in range(2)]
        Yout = kpool.tile([128, 8], dt.float32, tag="yout")

        hprev = [None] * NGROUP
        stash = [[] for _ in range(NGROUP)]

        def st_mm(G, l, side, nfts):
            p = G % 2
            zps = ps.tile([128, 512], dt.float32, tag=f"zmm{p}", name=f"z{G}{l}{side}")
            zv = zps[:].rearrange("p (c gf) -> p c gf", c=4)
            zv4 = zps[:].rearrange("p (c g f) -> p c g f", c=4, g=4)
            if l == 0 and side == "E":
                for g in range(4):
                    for c in range(4):
                        nc.tensor.matmul(zv4[:, c, g, :],
                                         nfts[g][:, 128 * c : 128 * c + 128],
                                         w0[:], start=True, stop=True,
                                         tile_position=(0, 0))
            else:
                wmov = bde[l - 1] if side == "E" else bdn[l]
                for c in range(4):
                    nc.tensor.matmul(zv[:, c, :],
                                     hprev[G][:, 128 * c : 128 * c + 128],
                                     wmov[:], start=True, stop=True,
                                     tile_position=(0, 0))
            return zps

        def st_zsplit(G, l, side, zps):
            zv4 = zps[:].rearrange("p (c g f) -> p c g f", c=4, g=4)
            zs = spool.tile([128, 4, 4, 64], dt.float16, tag="zs", name=f"zs{G}{l}{side}")
            nc.scalar.copy(zs[:, :, :, 0:32], zv4)
            nc.vector.tensor_tensor(zs[:, :, :, 32:64], zv4,
                                    zs[:, :, :, 0:32], ALU.subtract)
            return zs

        def st_agg(G, l, side, zs, As, Ats):
            p = G % 2
            mats = As if side == "E" else Ats
            pb01 = ps.tile([128, 512], dt.float32, tag=f"a01_{p}", name=f"pa{G}{l}{side}")
            pb23 = ps.tile([128, 512], dt.float32, tag=f"a23_{p}", name=f"pb{G}{l}{side}")
            for g in range(4):
                pb = pb01 if g < 2 else pb23
                col = 64 * (g % 2)
                for c in range(4):
                    nc.tensor.matmul(pb[col : col + 64, :],
                                     zs[:, c, g, :], mats[g][:, c, :],
                                     start=(c == 0), stop=(c == 3),
                                     tile_position=(0, col))
            return pb01, pb23

        def st_ssplit(G, l, side, pb01, pb23):
            # exact 3-component fp16 decomposition of both agg psums
            s01 = spool.tile([128, 3, 512], dt.float16, tag="s01", name=f"s01{G}{l}{side}")
            s23 = spool.tile([128, 3, 512], dt.float16, tag="s23", name=f"s23{G}{l}{side}")
            t01 = spool.tile([128, 512], dt.float32, tag="t01", name=f"t01{G}{l}{side}")
            t23 = spool.tile([128, 512], dt.float32, tag="t23", name=f"t23{G}{l}{side}")
            nc.scalar.copy(s01[:, 0, :], pb01[:])
            nc.vector.tensor_tensor(t01[:], pb01[:], s01[:, 0, :], ALU.subtract)
            nc.scalar.copy(s01[:, 1, :], t01[:])
            nc.gpsimd.tensor_tensor(s01[:, 2, :], t01[:], s01[:, 1, :], ALU.subtract)
            nc.scalar.copy(s23[:, 0, :], pb23[:])
            nc.vector.tensor_tensor(t23[:], pb23[:], s23[:, 0, :], ALU.subtract)
            nc.vector.tensor_copy(s23[:, 1, :], t23[:])
            nc.gpsimd.tensor_tensor(s23[:, 2, :], t23[:], s23[:, 1, :], ALU.subtract)
            return s01, s23

        def st_sel(G, l, side, s01, s23):
            p = G % 2
            u = ps.tile([128, 512], dt.float32, tag=f"u{p}", name=f"u{G}{l}{side}")
            for half, s3 in ((0, s01), (1, s23)):
                col = 64 * half
                for comp in range(3):
                    nc.tensor.matmul(u[col : col + 64, :], sel[:], s3[:, comp, :],
                                     start=(comp == 0), stop=(comp == 2),
                                     tile_position=(0, col))
            return u

        def st_act(G, l, side, u, recip):
            if side == "E":
                ue = spool.tile([128, 512], dt.float32, tag="ue", name=f"ue{G}{l}")
                nc.vector.scalar_tensor_tensor(ue[:], u[:], bepp[l][:],
                                               recip[:], ALU.add, ALU.mult)
                h32 = spool.tile([128, 512], dt.float32, tag="h32", name=f"he{G}{l}")
                nc.scalar.activation(h32[:], ue[:], AF.Tanh)
            else:
                h32 = hpool.tile([128, 512], dt.float32, tag=f"hc{l}", name=f"hn{G}{l}")
                nc.scalar.activation(h32[:], u[:], AF.Tanh, bias=bnpp[l][:],
                                     scale=1.0 / HDEG)
                stash[G].append(h32)
                if l == 3:
                    krows = h32[:].rearrange("(a b) e -> a b e", b=32)[:, 0, :]
                    r0 = 4 * (G % 2)
                    nc.sync.dma_start(keys8[G // 2][r0 : r0 + 4, :], krows)
            hprev[G] = h32

        for P in range(2):
            groups = (2 * P, 2 * P + 1)
            Ad, Atd, nfd, rcd = {}, {}, {}, {}
            for G in groups:
                p = G % 2
                As, Ats, nfts = [], [], []
                for g in range(4):
                    gl = 4 * G + g
                    a_t = apool.tile([128, 4, 512], dt.float8e4, tag=f"a{p}{g}",
                                     name=f"a{G}{g}")
                    nc.sync.dma_start(a_t[:], AD[gl])
                    As.append(a_t)
                    at_t = apool.tile([128, 4, 512], dt.float8e4, tag=f"t{p}{g}",
                                      name=f"t{G}{g}")
                    nc.sync.dma_start(at_t[:], ATD[gl])
                    Ats.append(at_t)
                    nt = gpool.tile([128, 512], dt.float32, tag=f"nf{p}{g}",
                                    name=f"n{G}{g}")
                    nc.sync.dma_start(nt[:], NFT[gl])
                    nfts.append(nt)
                rc = gpool.tile([128, 512], dt.float32, tag=f"rc{p}", name=f"rc{G}")
                nc.sync.dma_start(rc[:], RECIP[G])
                Ad[G], Atd[G], nfd[G], rcd[G] = As, Ats, nfts, rc

            for l in range(4):
                for side in ("E", "N"):
                    # stage-interleaved across the group pair so each
                    # engine's FIFO always holds independent work
                    zpsd = {G: st_mm(G, l, side, nfd[G]) for G in groups}
                    zsd = {G: st_zsplit(G, l, side, zpsd[G]) for G in groups}
                    pbd = {G: st_agg(G, l, side, zsd[G], Ad[G], Atd[G])
                           for G in groups}
                    s3d = {G: st_ssplit(G, l, side, *pbd[G]) for G in groups}
                    ud = {G: st_sel(G, l, side, *s3d[G]) for G in groups}
                    for G in groups:
                        st_act(G, l, side, ud[G], rcd[G])

            # ---- per-pair top-k (in place on keys8); IDXS write hides
            # under the next pair's compute ----
            kw = keys8[P]
            idxu = kpool.tile([8, 32], dt.uint32, tag=f"idxu{P}", name=f"idxu{P}")
            for r in range(4):
                m8 = kpool.tile([8, 8], dt.float32, tag=f"m8{P}", name=f"m8{P}")
                nc.vector.max(m8[:], kw[:])
                nc.vector.max_index(idxu[:, 8 * r : 8 * r + 8], m8[:], kw[:])
                nc.vector.match_replace(kw[:], m8[:], kw[:], -1e30)
            idx16 = kpool.tile([8, 32], dt.int16, tag=f"idx16{P}", name=f"idx16{P}")
            nc.vector.tensor_copy(idx16[:], idxu[:])
            nc.sync.dma_start(IDXS[8 * P : 8 * P + 8], idx16[:])

        # ---- pooled gather + conv tower, all groups (after both pairs) ----
        for G in range(NGROUP):
                p = G % 2
                idxw = tpool.tile([128, 2], dt.int16, tag=f"idxw{p}", name=f"ix{G}")
                for m in range(4):
                    src_m = IDXS[4 * G + m].rearrange("(t lo) -> lo t", lo=16)
                    for half in range(2):
                        base = 32 * m + 16 * half
                        nc.sync.dma_start(idxw[base : base + 16, :], src_m)

                pgs = []
                for l in range(4):
                    pg = tpool.tile([128, 32], dt.float32, tag=f"pg{p}{l}",
                                    name=f"pg{G}{l}")
                    nc.gpsimd.ap_gather(pg[:], stash[G][l][:].unsqueeze(2), idxw[:],
                                        channels=128, num_elems=512, d=1, num_idxs=32)
                    pgs.append(pg)

                y1 = ps.tile([128, 30], dt.float32, tag=f"u{p}", name=f"y1{G}")
                for g in range(4):
                    for l in range(4):
                        nc.tensor.matmul(y1[32 * g : 32 * g + 16, :],
                                         cw1[l][32 * g : 32 * g + 32, :],
                                         pgs[l][32 * g : 32 * g + 32, 0:30],
                                         start=(l == 0), stop=(l == 3),
                                         tile_position=(32 * g, 32 * g))
                y1r = tpool.tile([128, 30], dt.float32, tag=f"y1r{p}", name=f"y1r{G}")
                nc.scalar.activation(y1r[:], y1[:], AF.Relu, bias=cb1[:])
                y1p = tpool.tile([128, 15], dt.float32, tag=f"y1p{p}", name=f"y1p{G}")
                nc.vector.tensor_tensor(
                    y1p[:], y1r[:].rearrange("p (t two) -> p t two", two=2)[:, :, 0],
                    y1r[:].rearrange("p (t two) -> p t two", two=2)[:, :, 1], ALU.max)

                y2 = ps.tile([128, 11], dt.float32, tag=f"u{p}", name=f"y2{G}")
                for g in range(4):
                    for d in range(5):
                        nc.tensor.matmul(y2[32 * g : 32 * g + 32, :],
                                         cw2[d][32 * g : 32 * g + 32, :],
                                         y1p[32 * g : 32 * g + 32, d : d + 11],
                                         start=(d == 0), stop=(d == 4),
                                         tile_position=(32 * g, 32 * g))
                y2r = tpool.tile([128, 11], dt.float32, tag=f"y2r{p}", name=f"y2r{G}")
                nc.scalar.activation(y2r[:], y2[:], AF.Relu, bias=cb2[:])
                for o in range(2):
                    t_o = tpool.tile([128, 11], dt.float32, tag=f"t_o{p}", name=f"o{G}{o}")
                    nc.vector.tensor_tensor(t_o[:], y2r[:], ow[o][:], ALU.mult)
                    nc.vector.tensor_reduce(Yout[:, 2 * G + o : 2 * G + o + 1],
                                            t_o[:], AX.X, ALU.add)

        # ---- final dense + relu + output ----
        pout = ps.tile([4, 8], dt.float32, tag="u0", name="pout")
        nc.tensor.matmul(pout[:], ssum[:], Yout[:], start=True, stop=True)
        ob = kpool.tile([4, 8], dt.float32, tag="ob")
        nc.vector.tensor_tensor(ob[:], pout[:], outb[:], ALU.add)
        orl = kpool.tile([4, 8], dt.float32, tag="orl")
        nc.scalar.activation(orl[:], ob[:], AF.Relu)
        nc.sync.dma_start(OUT.rearrange("(G g) o -> g G o", g=4), orl[:])

    nc.compile()
    return nc


def _make_consts(inputs):
    ws = [np.asarray(inputs[f"w{i}"], np.float32) for i in range(8)]
    bs = [np.asarray(inputs[f"b{i}"], np.float32) for i in range(8)]
    wE = [ws[0], _pad32(ws[2]), _pad32(ws[4]), _pad32(ws[6])]
    wN = [_pad32(ws[1]), _pad32(ws[3]), _pad32(ws[5]), _pad32(ws[7])]
    bE = [bs[0], bs[2], bs[4], np.pad(bs[6], (0, 31))]
    bN = [bs[1], bs[3], bs[5], np.pad(bs[7], (0, 31))]

    bde = np.stack([_blockdiag4(wE[l]) for l in range(1, 4)])
    bdn = np.stack([_blockdiag4(wN[l]) for l in range(4)])
    bepp = np.stack([np.tile(bE[l], 4)[:, None] for l in range(4)]).astype(np.float32)
    bnpp = np.stack([(np.tile(bN[l], 4) / HDEG)[:, None] for l in range(4)]).astype(np.float32)

    sel = np.zeros((128, 64), np.float16)
    for g2 in range(2):
        for t in range(2):
            for f in range(32):
                sel[64 * g2 + 32 * t + f, 32 * g2 + f] = 1.0

    c1w = np.asarray(inputs["conv1_w"], np.float32).reshape(C1, 97)
    cw1 = np.zeros((4, 128, 16), np.float32)
    for l in range(4):
        blk = np.zeros((32, 16), np.float32)
        if l < 3:
            blk = c1w[:, 32 * l : 32 * l + 32].T
        else:
            blk[0, :] = c1w[:, 96]
        for g in range(4):
            cw1[l, 32 * g : 32 * g + 32, :] = blk
    cb1 = np.zeros((128,), np.float32)
    for g in range(4):
        cb1[32 * g : 32 * g + 16] = inputs["conv1_b"]
    c2w = np.asarray(inputs["conv2_w"], np.float32)
    cw2 = np.zeros((5, 128, 32), np.float32)
    for d in range(5):
        for g in range(4):
            cw2[d, 32 * g : 32 * g + 16, :] = c2w[:, :, d].T
    cb2 = np.zeros((128,), np.float32)
    for g in range(4):
        cb2[32 * g : 32 * g + 32] = inputs["conv2_b"]
    oww = np.asarray(inputs["out_w"], np.float32)
    ow = np.zeros((2, 128, 11), np.float32)
    for o in range(2):
        for g in range(4):
            ow[o, 32 * g : 32 * g + 32, :] = oww[:, o].reshape(C2, 11)
    outb = np.tile(np.asarray(inputs["out_b"], np.float32), (4, 4))
    ssum = np.zeros((128, 4), np.float32)
    for j in range(4):
        ssum[32 * j : 32 * j + 32, j] = 1.0

    return {
        "w0c": ws[0][:, :32].astype(np.float32),
        "bde": bde, "bdn": bdn,
        "bepp": bepp, "bnpp": bnpp, "sel": sel,
        "cw1": cw1, "cb1": cb1[:, None],
        "cw2": cw2, "cb2": cb2[:, None],
        "ow": ow, "outb": outb, "ssum": ssum,
    }


def _make_graph_data(inputs):
    inc_edge = np.asarray(inputs["inc_edge"]).reshape(B, NPER, DEG).astype(np.int64)
    base = (np.arange(B, dtype=np.int64) * EPER)[:, None, None]
    e_loc = inc_edge - base

    n_loc = np.arange(NPER, dtype=np.int64)[None, :, None]
    flat = (np.arange(B, dtype=np.int64)[:, None, None] * (NPER * EPER)
            + n_loc * EPER + e_loc).ravel()
    cnt = np.bincount(flat, minlength=B * NPER * EPER)
    Afull = cnt.reshape(B, NPER, EPER).astype(ml_dtypes.float8_e4m3fn)
    Adev = np.ascontiguousarray(
        Afull.reshape(B, 4, 128, EPER).transpose(0, 2, 1, 3))
    Atfull = Afull.transpose(0, 2, 1)
    Atdev = np.ascontiguousarray(
        Atfull.reshape(B, 4, 128, NPER).transpose(0, 2, 1, 3))

    nf = np.asarray(inputs["node_feat"], np.float32).reshape(B, NPER, F)
    nfT = np.ascontiguousarray(nf.transpose(0, 2, 1))

    ecnt = np.bincount(np.asarray(inputs["inc_edge"]).astype(np.int64).ravel(),
                       minlength=B * EPER).reshape(B, EPER)
    recip = (1.0 / (ecnt + 1.0)).astype(np.float32)
    return Adev, Atdev, nfT, recip


def get_program():
    if "nc" not in _CACHE:
        _CACHE["nc"] = _build_program()
    return _CACHE["nc"]


def make_in_maps(inputs):
    consts = _make_consts(inputs)
    Adev, Atdev, nfT, recip = _make_graph_data(inputs)
    in_maps = []
    for core in range(NCORES):
        lo, hi = core * GPC, (core + 1) * GPC
        m = dict(consts)
        m["amat"] = np.ascontiguousarray(Adev[lo:hi])
        m["atmat"] = np.ascontiguousarray(Atdev[lo:hi])
        m["nft"] = np.ascontiguousarray(nfT[lo:hi])
        r = recip[lo:hi].reshape(NGROUP, 4, 1, EPER)
        m["recip"] = np.ascontiguousarray(
            np.broadcast_to(r, (NGROUP, 4, 32, EPER)).reshape(NGROUP, 128, EPER))
        in_maps.append(m)
    return in_maps


def kernel(**inputs):
    nc = get_program()
    in_maps = make_in_maps(inputs)
    res = run_bass_kernel_spmd(nc, in_maps, core_ids=list(range(NCORES)))
    out = np.concatenate([res.results[c]["out"] for c in range(NCORES)], axis=0)
    return out.astype(np.float32)
